# revision 8
# baseline (speedup 1.0000x reference)
"""DiT block kernel for 8 TRN2 NeuronCores (data-parallel over batch).

Each core processes one batch element b of x[8, 1024, 1024], c[8, 1024].
Weights are replicated. Activations are kept feature-major ("transposed",
features on SBUF partitions) so every projection uses naturally-laid-out
weights as the stationary matmul operand and no activation transposes are
needed. Attention computes S^T = K.QT directly (j on partitions) so the
softmax sum folds into the P.V matmul via an extra ones-column appended to
the V projection; normalization happens after P.V with a per-token
reciprocal broadcast.

All matmuls run in bf16 with fp32 PSUM accumulation; LayerNorm statistics,
softmax and residuals are fp32.
"""

import sys

import numpy as np

if "/opt/trn_rl_repo" not in sys.path:
    sys.path.insert(0, "/opt/trn_rl_repo")

import ml_dtypes

B, T, HID, NH, HD, MLP = 8, 1024, 1024, 16, 64, 4096
P = 128
KT = HID // P  # 8 k-tiles over hidden dim
MT = MLP // P  # 32 m-tiles over mlp dim
VAUG = NH * (HD + 1)  # 1040: per-head 64 v columns + 1 ones column
EPS = 1e-6
BF = ml_dtypes.bfloat16

N_CORES = 8

_CACHE = {}


def _ln_modulate(nc, get_src, dst, scp1, scfull, sc_col, sh_col, ones_bf,
                 bfs, rows, bcast, tmp, ps_st):
    """dst[:, k, :] = ((src - mu) * rsigma) * (1 + sc[k]) + sh[k]  (bf16).

    get_src(k) -> [128, T] fp32 AP (feature-major k-tile). Called twice per
    k (stats pass + modulate pass). Stats over the feature dim (partitions
    x k-tiles) via ones-matmuls on the PE.
    """
    import concourse.mybir as mybir
    from concourse.bass import ts
    f32 = mybir.dt.float32
    bf16 = mybir.dt.bfloat16
    AF = mybir.ActivationFunctionType
    OP = mybir.AluOpType

    mu_row = rows.tile([1, T], f32, name="mu_row", tag="rows")
    msq_row = rows.tile([1, T], f32, name="msq_row", tag="rows")
    ps_mus = [ps_st.tile([1, 512], f32, name="ps_mu", tag="ps_st")
              for _ in range(2)]
    ps_sqs = [ps_st.tile([1, 512], f32, name="ps_sq", tag="ps_st")
              for _ in range(2)]
    for k in range(KT):
        src = get_src(k)
        xbf = bfs.tile([P, T], bf16, name="xbf", tag="xbf")
        nc.vector.tensor_copy(out=xbf[:], in_=src)
        xsq = bfs.tile([P, T], bf16, name="xsq", tag="xsq")
        nc.vector.tensor_mul(xsq[:], xbf[:], xbf[:])
        for th in range(2):
            nc.tensor.matmul(ps_mus[th][:], ones_bf[:], xbf[:, ts(th, 512)],
                             start=(k == 0), stop=(k == KT - 1))
            nc.tensor.matmul(ps_sqs[th][:], ones_bf[:], xsq[:, ts(th, 512)],
                             start=(k == 0), stop=(k == KT - 1))
    for th in range(2):
        nc.scalar.activation(mu_row[0:1, ts(th, 512)], ps_mus[th][:], AF.Copy,
                             scale=1.0 / HID)
        nc.scalar.activation(msq_row[0:1, ts(th, 512)], ps_sqs[th][:],
                             AF.Copy, scale=1.0 / HID)
    var_row = rows.tile([1, T], f32, name="var_row", tag="rows")
    nc.vector.tensor_mul(var_row[:], mu_row[:], mu_row[:])
    nc.vector.tensor_sub(var_row[:], msq_row[:], var_row[:])
    eps_row = rows.tile([1, 1], f32, name="eps_row", tag="eps")
    nc.vector.memset(eps_row[:], EPS)
    sd_row = rows.tile([1, T], f32, name="sd_row", tag="rows")
    nc.scalar.activation(sd_row[:], var_row[:], AF.Sqrt, bias=eps_row[:])
    rs_row = rows.tile([1, T], f32, name="rs_row", tag="rows")
    nc.vector.reciprocal(rs_row[:], sd_row[:])
    nm_row = rows.tile([1, T], f32, name="nm_row", tag="rows")
    nc.vector.tensor_mul(nm_row[:], mu_row[:], rs_row[:])
    nc.vector.tensor_scalar_mul(nm_row[:], nm_row[:], -1.0)

    rs_b = bcast.tile([P, T], f32, name="rs_b", tag="bcast")
    nc.gpsimd.partition_broadcast(rs_b[:], rs_row[:])
    nm_b = bcast.tile([P, T], f32, name="nm_b", tag="bcast")
    nc.gpsimd.partition_broadcast(nm_b[:], nm_row[:])

    for k in range(KT):
        src = get_src(k)
        t = tmp.tile([P, T], f32, name="tmp", tag="tmp")
        nc.vector.tensor_mul(t[:], src, rs_b[:])
        nc.vector.tensor_add(t[:], t[:], nm_b[:])
        nc.vector.tensor_scalar(dst[:, k, :], t[:],
                                scp1[:, sc_col + k:sc_col + k + 1],
                                scfull[:, sh_col + k:sh_col + k + 1],
                                OP.mult, OP.add)


def build_nc():
    """Build + compile the single-core SPMD program. Cached."""
    if "nc" in _CACHE:
        return _CACHE["nc"]

    import concourse.bacc as bacc
    import concourse.mybir as mybir
    import concourse.tile as tile
    from concourse.bass import ts

    f32 = mybir.dt.float32
    bf16 = mybir.dt.bfloat16
    AF = mybir.ActivationFunctionType
    OP = mybir.AluOpType

    nc = bacc.Bacc("TRN2", target_bir_lowering=False, debug=False,
                   num_devices=N_CORES)

    # ---- DRAM tensors (names = in_map keys) ----
    d_xt = nc.dram_tensor("xt", (P, KT, T), f32, kind="ExternalInput")
    d_cp = nc.dram_tensor("cp", (P, KT), f32, kind="ExternalInput")
    d_wada = nc.dram_tensor("wada", (P, KT, 6 * HID), bf16, kind="ExternalInput")
    d_bada = nc.dram_tensor("bada", (P, 48), f32, kind="ExternalInput")
    d_wq = nc.dram_tensor("wq", (P, KT, KT, P), bf16, kind="ExternalInput")
    d_wk = nc.dram_tensor("wk", (P, KT, KT, P), bf16, kind="ExternalInput")
    d_wo = nc.dram_tensor("wo", (P, KT, KT, P), bf16, kind="ExternalInput")
    d_bqt = nc.dram_tensor("bqt", (P, KT), f32, kind="ExternalInput")
    d_bkt = nc.dram_tensor("bkt", (P, KT), f32, kind="ExternalInput")
    d_bot = nc.dram_tensor("bot", (P, KT), f32, kind="ExternalInput")
    d_wv = nc.dram_tensor("wv", (P, KT, VAUG), bf16, kind="ExternalInput")
    d_bv = nc.dram_tensor("bv", (1, VAUG), f32, kind="ExternalInput")
    d_w1 = nc.dram_tensor("w1", (P, MT, KT, P), bf16, kind="ExternalInput")
    d_b1t = nc.dram_tensor("b1t", (P, MT), f32, kind="ExternalInput")
    d_w2 = nc.dram_tensor("w2", (P, KT, MT, P), bf16, kind="ExternalInput")
    d_b2t = nc.dram_tensor("b2t", (P, KT), f32, kind="ExternalInput")
    d_y = nc.dram_tensor("y", (HID, T), f32, kind="ExternalOutput")

    with tile.TileContext(nc) as tc:
        with tc.tile_pool(name="const", bufs=1) as const, \
             tc.tile_pool(name="act", bufs=2) as act, \
             tc.tile_pool(name="y1p", bufs=1) as y1p, \
             tc.tile_pool(name="yout", bufs=3) as yout, \
             tc.tile_pool(name="tmp", bufs=3) as tmp:

            # ---------- global constants ----------
            bqt_sb = const.tile([P, KT], f32, name="bqt_sb")
            nc.sync.dma_start(bqt_sb[:], d_bqt.ap())
            bkt_sb = const.tile([P, KT], f32, name="bkt_sb")
            nc.sync.dma_start(bkt_sb[:], d_bkt.ap())
            bot_sb = const.tile([P, KT], f32, name="bot_sb")
            nc.sync.dma_start(bot_sb[:], d_bot.ap())
            b1t_sb = const.tile([P, MT], f32, name="b1t_sb")
            nc.sync.dma_start(b1t_sb[:], d_b1t.ap())
            b2t_sb = const.tile([P, KT], f32, name="b2t_sb")
            nc.sync.dma_start(b2t_sb[:], d_b2t.ap())
            ones_bf = const.tile([P, 1], bf16, name="ones_bf")
            nc.vector.memset(ones_bf[:], 1.0)
            one11 = const.tile([1, 1], f32, name="one11")
            nc.vector.memset(one11[:], 1.0)
            scfull = const.tile([P, 48], f32, name="scfull")
            scp1 = const.tile([P, 48], f32, name="scp1")

            xm = act.tile([P, KT, T], bf16, name="xm", tag="act")

            # ---------- scope A: adaLN vector + LN1/modulate ----------
            with tc.tile_pool(name="wadap", bufs=4) as wadap, \
                 tc.tile_pool(name="rowada", bufs=3) as rowada, \
                 tc.tile_pool(name="xstrA", bufs=3) as xstrA, \
                 tc.tile_pool(name="bfsA", bufs=3) as bfsA, \
                 tc.tile_pool(name="rowsA", bufs=3) as rowsA, \
                 tc.tile_pool(name="bcastA", bufs=2) as bcastA, \
                 tc.tile_pool(name="ps_ada", bufs=2, space="PSUM") as ps_ada, \
                 tc.tile_pool(name="ps_tr", bufs=1, space="PSUM") as ps_tr, \
                 tc.tile_pool(name="ps_st", bufs=4, space="PSUM") as ps_st:
                ct = rowada.tile([P, KT], f32, name="ct", tag="ct")
                nc.sync.dma_start(ct[:], d_cp.ap())
                silu_bf = const.tile([P, KT], bf16, name="silu_bf")
                nc.scalar.activation(silu_bf[:], ct[:], AF.Silu)

                pst = ps_tr.tile([P, 48], f32, name="ps_tr")
                for n in range(12):
                    ps = ps_ada.tile([1, 512], f32, name="ps_ada")
                    for k in range(KT):
                        wsl = wadap.tile([P, 512], bf16, name="wsl")
                        nc.sync.dma_start(wsl[:], d_wada.ap()[:, k, ts(n, 512)])
                        nc.tensor.matmul(ps[:], silu_bf[:, k:k + 1], wsl[:],
                                         start=(k == 0), stop=(k == KT - 1))
                    row_n = rowada.tile([1, 512], f32, name="row_n", tag="row")
                    nc.scalar.activation(row_n[:], ps[:], AF.Copy)
                    # scatter [1, 512] into 4 columns of [128, 48] via K=1 mm
                    for jj in range(4):
                        j = n * 4 + jj
                        nc.tensor.matmul(pst[:, j:j + 1],
                                         row_n[0:1, ts(jj, P)], one11[:],
                                         start=True, stop=True)
                bada_sb = rowada.tile([P, 48], f32, name="bada_sb", tag="bada")
                nc.sync.dma_start(bada_sb[:], d_bada.ap())
                nc.vector.tensor_add(scfull[:], pst[:], bada_sb[:])
                nc.vector.tensor_scalar_add(scp1[:], scfull[:], 1.0)

                def src_x(k):
                    t = xstrA.tile([P, T], f32, name="xstr", tag="xstr")
                    nc.sync.dma_start(t[:], d_xt.ap()[:, k, :])
                    return t[:]

                _ln_modulate(nc, src_x, xm, scp1, scfull, 8, 0, ones_bf,
                             bfsA, rowsA, bcastA, tmp, ps_st)

            # ---------- scope B: QKV + attention + out-proj ----------
            y1 = y1p.tile([P, KT, T], f32, name="y1")
            ps_mm_cm = tc.tile_pool(name="ps_mm", bufs=4, space="PSUM")
            ps_mm = ps_mm_cm.__enter__()
            with tc.tile_pool(name="qk", bufs=2) as qk, \
                 tc.tile_pool(name="vpool", bufs=1) as vpool, \
                 tc.tile_pool(name="wvp", bufs=1) as wvp, \
                 tc.tile_pool(name="wsm", bufs=4) as wsm, \
                 tc.tile_pool(name="epool", bufs=3) as epool, \
                 tc.tile_pool(name="rpool", bufs=2) as rpool, \
                 tc.tile_pool(name="xstrB", bufs=3) as xstrB, \
                 tc.tile_pool(name="ps_s", bufs=2, space="PSUM") as ps_s, \
                 tc.tile_pool(name="ps_o", bufs=2, space="PSUM") as ps_o:
                wv_sb = wvp.tile([P, KT, VAUG], bf16, name="wv_sb", tag="wv")
                for g in range(4):
                    nc.sync.dma_start(wv_sb[:, ts(g, 2)], d_wv.ap()[:, ts(g, 2)])
                bv_row = rpool.tile([1, VAUG], f32, name="bv_row", tag="bvr")
                nc.sync.dma_start(bv_row[:], d_bv.ap())
                bvb = wvp.tile([P, VAUG], f32, name="bvb", tag="bvb")
                nc.gpsimd.partition_broadcast(bvb[:], bv_row[:])

                # v token-major with aug ones-column: [128, 8 tb, 1040]
                v_sb = vpool.tile([P, KT, VAUG], bf16, name="v_sb")
                for tb in range(KT):
                    for (ns, nw) in ((0, 512), (512, 512), (1024, VAUG - 1024)):
                        psv = ps_mm.tile([P, 512], f32, name="ps_v",
                                         tag="ps_mm")
                        for k in range(KT):
                            nc.tensor.matmul(psv[:, 0:nw], xm[:, k, ts(tb, P)],
                                             wv_sb[:, k, ns:ns + nw],
                                             start=(k == 0),
                                             stop=(k == KT - 1))
                        nc.vector.tensor_add(v_sb[:, tb, ns:ns + nw],
                                             psv[:, 0:nw], bvb[:, ns:ns + nw])

                # q/k feature-major [128, KT, T] (2 heads per f-tile)
                qT = qk.tile([P, KT, T], bf16, name="qT", tag="qk")
                kT = qk.tile([P, KT, T], bf16, name="kT", tag="qk")
                for (d_w, b_sb, oT) in ((d_wq, bqt_sb, qT), (d_wk, bkt_sb, kT)):
                    for m in range(KT):
                        wsl = wsm.tile([P, KT, P], bf16, name="wsl_qk",
                                       tag="wsm")
                        nc.sync.dma_start(wsl[:], d_w.ap()[:, m])
                        ps0 = ps_mm.tile([P, 512], f32, name="ps_p0",
                                         tag="ps_mm")
                        ps1 = ps_mm.tile([P, 512], f32, name="ps_p1",
                                         tag="ps_mm")
                        for k in range(KT):
                            nc.tensor.matmul(ps0[:], wsl[:, k, :],
                                             xm[:, k, 0:512],
                                             start=(k == 0),
                                             stop=(k == KT - 1))
                            nc.tensor.matmul(ps1[:], wsl[:, k, :],
                                             xm[:, k, 512:T],
                                             start=(k == 0),
                                             stop=(k == KT - 1))
                        for th, ps in ((0, ps0), (1, ps1)):
                            nc.scalar.activation(oT[:, m, ts(th, 512)], ps[:],
                                                 AF.Identity,
                                                 bias=b_sb[:, m:m + 1])

                # attention
                attnT = act.tile([P, KT, T], bf16, name="attnT", tag="act")
                for h in range(NH):
                    ft, fr = h // 2, (h % 2) * HD
                    for ih in range(2):
                        pso = ps_o.tile([HD + 1, 512], f32, name="ps_o")
                        for jb in range(KT):
                            pss = ps_s.tile([P, 512], f32, name="ps_s")
                            nc.tensor.matmul(pss[:],
                                             kT[fr:fr + HD, ft, ts(jb, P)],
                                             qT[fr:fr + HD, ft, ts(ih, 512)],
                                             start=True, stop=True)
                            e_sb = epool.tile([P, 512], bf16, name="e_sb")
                            nc.scalar.activation(e_sb[:], pss[:], AF.Exp,
                                                 scale=1.0 / HD)
                            nc.tensor.matmul(pso[:],
                                             v_sb[:, jb, h * 65:h * 65 + 65],
                                             e_sb[:], start=(jb == 0),
                                             stop=(jb == KT - 1))
                        rec = rpool.tile([1, 512], f32, name="rec", tag="rec")
                        nc.vector.reciprocal(rec[:], pso[HD:HD + 1, :])
                        recB = rpool.tile([HD, 512], f32, name="recB",
                                          tag="recB")
                        nc.gpsimd.partition_broadcast(recB[:], rec[:])
                        nc.vector.tensor_mul(
                            attnT[fr:fr + HD, ft, ts(ih, 512)],
                            pso[0:HD, :], recB[:])

                # out-projection + residual 1
                for f in range(KT):
                    wsl = wsm.tile([P, KT, P], bf16, name="wsl_qk", tag="wsm")
                    nc.sync.dma_start(wsl[:], d_wo.ap()[:, f])
                    ps0 = ps_mm.tile([P, 512], f32, name="ps_p0", tag="ps_mm")
                    ps1 = ps_mm.tile([P, 512], f32, name="ps_p1", tag="ps_mm")
                    for k in range(KT):
                        nc.tensor.matmul(ps0[:], wsl[:, k, :],
                                         attnT[:, k, 0:512],
                                         start=(k == 0), stop=(k == KT - 1))
                        nc.tensor.matmul(ps1[:], wsl[:, k, :],
                                         attnT[:, k, 512:T],
                                         start=(k == 0), stop=(k == KT - 1))
                    xf = xstrB.tile([P, T], f32, name="xf", tag="xstr")
                    nc.sync.dma_start(xf[:], d_xt.ap()[:, f, :])
                    for th, ps in ((0, ps0), (1, ps1)):
                        t2 = tmp.tile([P, T], f32, name="tmp", tag="tmp")
                        nc.vector.tensor_scalar(t2[:, 0:512], ps[:],
                                                bot_sb[:, f:f + 1],
                                                scfull[:, 16 + f:17 + f],
                                                OP.add, OP.mult)
                        nc.vector.tensor_add(y1[:, f, ts(th, 512)],
                                             t2[:, 0:512],
                                             xf[:, ts(th, 512)])

            # ---------- scope C: LN2 + modulate ----------
            xm2 = act.tile([P, KT, T], bf16, name="xm2", tag="act")
            with tc.tile_pool(name="bfsC", bufs=3) as bfsC, \
                 tc.tile_pool(name="rowsC", bufs=3) as rowsC, \
                 tc.tile_pool(name="bcastC", bufs=2) as bcastC, \
                 tc.tile_pool(name="ps_st2", bufs=4, space="PSUM") as ps_st2:
                _ln_modulate(nc, lambda k: y1[:, k, :], xm2, scp1, scfull,
                             32, 24, ones_bf, bfsC, rowsC, bcastC, tmp,
                             ps_st2)

            # ---------- scope D: MLP + residual 2 ----------
            with tc.tile_pool(name="hpool", bufs=1) as hpool, \
                 tc.tile_pool(name="w1p", bufs=4) as w1p, \
                 tc.tile_pool(name="w2p", bufs=2) as w2p:
                h_sb = hpool.tile([P, MT, T], bf16, name="h_sb")
                for m in range(MT):
                    w1c = w1p.tile([P, KT, P], bf16, name="w1c")
                    nc.sync.dma_start(w1c[:], d_w1.ap()[:, m])
                    ps0 = ps_mm.tile([P, 512], f32, name="ps_p0", tag="ps_mm")
                    ps1 = ps_mm.tile([P, 512], f32, name="ps_p1", tag="ps_mm")
                    for k in range(KT):
                        nc.tensor.matmul(ps0[:], w1c[:, k, :], xm2[:, k, 0:512],
                                         start=(k == 0), stop=(k == KT - 1))
                        nc.tensor.matmul(ps1[:], w1c[:, k, :], xm2[:, k, 512:T],
                                         start=(k == 0), stop=(k == KT - 1))
                    nc.scalar.activation(h_sb[:, m, 0:512], ps0[:], AF.Gelu,
                                         bias=b1t_sb[:, m:m + 1])
                    nc.scalar.activation(h_sb[:, m, 512:T], ps1[:], AF.Gelu,
                                         bias=b1t_sb[:, m:m + 1])
                for o in range(KT):
                    w2c = w2p.tile([P, MT, P], bf16, name="w2c")
                    nc.sync.dma_start(w2c[:, 0:MT // 2],
                                      d_w2.ap()[:, o, 0:MT // 2])
                    nc.sync.dma_start(w2c[:, MT // 2:MT],
                                      d_w2.ap()[:, o, MT // 2:MT])
                    ps0 = ps_mm.tile([P, 512], f32, name="ps_p0", tag="ps_mm")
                    ps1 = ps_mm.tile([P, 512], f32, name="ps_p1", tag="ps_mm")
                    for m in range(MT):
                        nc.tensor.matmul(ps0[:], w2c[:, m, :], h_sb[:, m, 0:512],
                                         start=(m == 0), stop=(m == MT - 1))
                        nc.tensor.matmul(ps1[:], w2c[:, m, :], h_sb[:, m, 512:T],
                                         start=(m == 0), stop=(m == MT - 1))
                    for th, ps in ((0, ps0), (1, ps1)):
                        yt = yout.tile([P, 512], f32, name="yt")
                        nc.vector.tensor_scalar(yt[:], ps[:],
                                                b2t_sb[:, o:o + 1],
                                                scfull[:, 40 + o:41 + o],
                                                OP.add, OP.mult)
                        nc.vector.tensor_add(yt[:], yt[:], y1[:, o, ts(th, 512)])
                        nc.sync.dma_start(d_y.ap()[ts(o, P), ts(th, 512)], yt[:])
            ps_mm_cm.__exit__(None, None, None)

    nc.compile()
    _CACHE["nc"] = nc
    return nc


def prep_in_maps(x, c, w_ada, b_ada, wq, bq, wk, bk, wv, bv, wo, bo,
                 w1, b1, w2, b2):
    """Host-side sharding + layout packing. Returns one in_map per core."""
    def lhsT_pack(W, kt, mt):
        # W [K, M] -> [128, mt, kt, 128]; slice [:, m, k, :] = W-tile (k, m)
        return np.ascontiguousarray(
            np.asarray(W, np.float32).reshape(kt, P, mt, P)
            .transpose(1, 2, 0, 3)).astype(BF)

    def rhs_pack(W):
        # W [K, F] -> [128, K//128, F]
        K, F = W.shape
        return np.ascontiguousarray(
            np.asarray(W, np.float32).reshape(K // P, P, F)
            .transpose(1, 0, 2)).astype(BF)

    def col_pack(v, n):
        return np.ascontiguousarray(np.asarray(v, np.float32).reshape(n, P).T)

    x = np.asarray(x, np.float32)
    c = np.asarray(c, np.float32)
    wv_aug = np.zeros((HID, VAUG), np.float32)
    bv_aug = np.zeros((1, VAUG), np.float32)
    wv = np.asarray(wv, np.float32)
    bv = np.asarray(bv, np.float32)
    for h in range(NH):
        wv_aug[:, h * 65:h * 65 + HD] = wv[:, h * HD:(h + 1) * HD]
        bv_aug[0, h * 65:h * 65 + HD] = bv[h * HD:(h + 1) * HD]
        bv_aug[0, h * 65 + HD] = 1.0

    shared = {
        "wada": rhs_pack(np.asarray(w_ada, np.float32)),
        "bada": np.ascontiguousarray(
            np.asarray(b_ada, np.float32).reshape(6, KT, P)
            .transpose(2, 0, 1).reshape(P, 48)),
        "wq": lhsT_pack(wq, KT, KT),
        "wk": lhsT_pack(wk, KT, KT),
        "wo": lhsT_pack(wo, KT, KT),
        "bqt": col_pack(bq, KT),
        "bkt": col_pack(bk, KT),
        "bot": col_pack(bo, KT),
        "wv": rhs_pack(wv_aug),
        "bv": bv_aug,
        "w1": lhsT_pack(w1, KT, MT),
        "b1t": col_pack(b1, MT),
        "w2": lhsT_pack(w2, MT, KT),
        "b2t": col_pack(b2, KT),
    }
    in_maps = []
    for b in range(B):
        m = dict(shared)
        m["xt"] = np.ascontiguousarray(
            x[b].T.reshape(KT, P, T).transpose(1, 0, 2))
        m["cp"] = np.ascontiguousarray(c[b].reshape(KT, P).T)
        in_maps.append(m)
    return in_maps


def run(in_maps, trace=False, tmpdir=None):
    from concourse import bass_utils
    nc = build_nc()
    return bass_utils.run_bass_kernel_spmd(
        nc, in_maps, core_ids=list(range(N_CORES)), trace=trace,
        tmpdir=tmpdir)


def kernel(**inputs) -> np.ndarray:
    in_maps = prep_in_maps(**inputs)
    res = run(in_maps)
    out = np.stack([np.asarray(res.results[b]["y"]).T for b in range(B)])
    return np.ascontiguousarray(out.astype(np.float32))
